# revision 10
# baseline (speedup 1.0000x reference)
"""Trainium2 Bass kernel: batched int8 dequant-BMM.

out[b] = (x[b].f32 - a_zp) @ (y[b].f32 - b_zp) * alpha
  x: [96, 1024, 64] int8, y: [96, 64, 1024] int8 -> out: [96, 1024, 1024] f32

Sharding: batch dim 96 -> 12 per core across 8 cores (pure data parallel).

The kernel is HBM-store-roofline bound: 12x1024x1024 output elems/core.
Output is alpha * K with K an exact integer < 2^21, so bf16 rounding of
the final value has rel err <= 2^-8 ~ 4e-3 (vs the 2e-2 gate): store
bf16 (25.2 MB/core, ~70us at the 358 GB/s HBM-per-NC limit) instead of
f32 and upcast on host.

The co-critical resource is PSUM->SBUF copy throughput - only ACT and
DVE can read PSUM (measured ~1.34us / ~1.47us per [128,1024] fp32
copy incl overhead; no 16-bit speedup possible with fp32 PSUM source,
and fp32 PSUM is mandatory on TRN2). 96 copies across 2 engines ~=
the store stream, so everything else is kept off ACT/DVE:
  - x is pre-transposed ON HOST (layout only) to [B, 64(d), 8(r),
    128(p)] so lhsT tiles come straight from DRAM - no on-device
    transpose pipeline at all.
  - pairs 1-5 load via SWDGE cast-DMA (int8 DRAM -> bf16 SBUF, cast in
    the DMA engine), so their zero-point subtract is a cheap all-bf16
    ACT activation (~0.6us); pair 0 loads via HWDGE (which starts ~1us
    earlier than SWDGE behind the ~7us engine preamble) as raw int8
    with 1x DVE dequants, to get the first store out ASAP.
  - GpSimd tensor ops were measured at 14.7us per dequant - unusable.
Steady state per pair (2 batches, 16 m-tile copies, 4.2 MB stored):
stores 11.7us, ACT 8 copies + 2 deqs ~= 11.9us, DVE 8 copies ~= 11.8us,
PE ~7us: copies and stores are balanced within ~2%. gsize=2 stores
(0.5 MB) keep the copy->store coupling slack small; 4 PSUM tiles
(8 banks) ring matmuls ahead of copies.
"""

import numpy as np

B, S, D = 96, 1024, 64
N_CORES = 8
BPC = B // N_CORES  # batches per core = 12
NPAIRS = BPC // 2

_cache = {}


def _build(az: float, bz: float, al: float):
    key = (az, bz, al)
    if key in _cache:
        return _cache[key]

    from contextlib import ExitStack

    import concourse.mybir as mybir
    import concourse.tile as tile
    from concourse import bacc

    f32 = mybir.dt.float32
    bf16 = mybir.dt.bfloat16
    i8 = mybir.dt.int8
    AF = mybir.ActivationFunctionType

    nc = bacc.Bacc(
        "TRN2", target_bir_lowering=False, debug=False, num_devices=N_CORES
    )
    # x arrives host-pre-transposed as [b, d, r, p] with s = 8p + r
    x_d = nc.dram_tensor("x", [BPC, D, 8, 128], i8, kind="ExternalInput").ap()
    y_d = nc.dram_tensor("y", [BPC, D, S], i8, kind="ExternalInput").ap()
    o_d = nc.dram_tensor("out", [BPC, S, S], bf16, kind="ExternalOutput").ap()

    # x[2c+bt, d, r, p] -> xv[bt*64+d, c, r, p]  (1KB runs per partition)
    xv = x_d.rearrange("(c b2) d r p -> (b2 d) c r p", b2=2)
    # y[2c+bt, d, s] -> yv[bt*64+d, c, s]  (contiguous in DRAM)
    yv = y_d.rearrange("(c b2) d s -> (b2 d) c s", b2=2)
    # out[b, 8p+r, t] <- ovn[b, p, r, t]: the row-residue m-tiling makes
    # the store rows of one partition contiguous in DRAM (gsize*2KB runs)
    ovn = o_d.rearrange("b (p r) t -> b p r t", p=128, r=8)

    GSIZE = 2  # r-tiles per store: small stores = tight store pipelining

    with tile.TileContext(nc) as tc, ExitStack() as ctx:
        const_pool = ctx.enter_context(tc.tile_pool(name="const", bufs=1))
        xin_pool = ctx.enter_context(tc.tile_pool(name="xin", bufs=NPAIRS))
        yin_pool = ctx.enter_context(tc.tile_pool(name="yin", bufs=1))
        y0_pool = ctx.enter_context(tc.tile_pool(name="y0", bufs=1))
        xt_pool = ctx.enter_context(tc.tile_pool(name="xt", bufs=3))
        ybf_pool = ctx.enter_context(tc.tile_pool(name="ybf", bufs=3))
        stage_pool = ctx.enter_context(tc.tile_pool(name="stage", bufs=12))
        mpsum_pool = ctx.enter_context(
            tc.tile_pool(name="mpsum", bufs=4, space="PSUM")
        )

        import ml_dtypes

        # Pair 0 rides HWDGE as raw int8, first thing on the sync and
        # scalar queues - these issue right after the engine preamble
        # (~6us), about 2us before the gpsimd (SWDGE) queue gets going.
        y_sb = yin_pool.tile([128, NPAIRS, S], bf16)
        x0 = xin_pool.tile([128, 8, 128], i8, tag="x2")
        y0 = y0_pool.tile([128, S], i8)
        nc.sync.dma_start(out=x0[:], in_=xv[:, 0])
        nc.scalar.dma_start(out=y0[:], in_=yv[:, 0, :])
        x2s = [x0]

        # Minimal HAM warmup: a few matmuls to flip the PE clock gate
        # from 1.2 to 2.4 GHz. (The old 8-matmul warmup at cold clock
        # blocked the real matmul stream for ~3.5us - PE FIFO.)
        warm_dram = nc.inline_tensor(
            np.ones((128, 512), dtype=ml_dtypes.bfloat16), name="warm512"
        ).ap()
        warm_sb = const_pool.tile([128, 512], bf16)
        nc.sync.dma_start(out=warm_sb[:], in_=warm_dram)
        warm_ps = mpsum_pool.tile([128, S], f32, tag="mpsum")
        for w in range(3):
            nc.tensor.matmul(
                warm_ps[:, (w % 2) * 512 : (w % 2 + 1) * 512],
                warm_sb[:, :128],
                warm_sb[:],
                start=True,
                stop=True,
            )

        # Pairs 1-5 ride SWDGE cast-DMA (int8 DRAM -> bf16 SBUF; only
        # SWDGE casts) up front into the pre-store DMA window.
        for c in range(1, NPAIRS):
            x2 = xin_pool.tile([128, 8, 128], bf16, tag="x2")
            nc.gpsimd.dma_start(out=x2[:], in_=xv[:, c])
            nc.gpsimd.dma_start(out=y_sb[:, c, :], in_=yv[:, c, :])
            x2s.append(x2)

        # Zero-point subtract. Pair 0: DVE 1x from int8 (DVE is idle
        # before its copy stream starts). Pairs 1-5: ACT bf16
        # activations (~0.6us each; ACT has no port sharing, so SWDGE
        # descriptor generation is never starved by perf-mode locks).
        preps = {}

        def prep(c):
            xt = xt_pool.tile([128, 8, 128], bf16, tag="xt")
            y2bf = ybf_pool.tile([128, S], bf16, tag="y2bf")
            if c == 0:
                nc.vector.tensor_scalar_add(xt[:], x2s[0][:], -az)
                nc.vector.tensor_scalar_add(y2bf[:], y0[:], -bz)
            else:
                nc.scalar.activation(
                    out=xt[:], in_=x2s[c][:], func=AF.Copy, bias=-az, scale=1.0
                )
                nc.scalar.activation(
                    out=y2bf[:], in_=y_sb[:, c, :], func=AF.Copy, bias=-bz, scale=1.0
                )
            preps[c] = (xt, y2bf)

        prep(0)
        prep(1)

        for c in range(NPAIRS):
            xt, y2bf = preps.pop(c)
            # e (bt=0, PE rows 0-63) and o (bt=1, rows 64-127) matmuls
            # issue adjacently so the row-tiled PE runs them concurrently.
            for g in range(8 // GSIZE):
                stages = []
                for bt in range(2):
                    stg = stage_pool.tile([128, GSIZE, S], bf16, tag="stage")
                    stages.append(stg)
                for j in range(GSIZE):
                    m = g * GSIZE + j
                    pss = []
                    for bt in range(2):
                        ps = mpsum_pool.tile([128, S], f32, tag="mpsum")
                        pss.append(ps)
                    for nh in range(2):
                        for bt in range(2):
                            nc.tensor.matmul(
                                pss[bt][:, nh * 512 : (nh + 1) * 512],
                                xt[bt * 64 : (bt + 1) * 64, m, :],
                                y2bf[bt * 64 : (bt + 1) * 64, nh * 512 : (nh + 1) * 512],
                                start=True,
                                stop=True,
                                tile_position=(bt * 64, 0),
                            )
                    # copies split 8:8 by parity, strictly interleaved
                    for bt in range(2):
                        if (m * 2 + bt) % 2 == 0:
                            nc.scalar.activation(
                                out=stages[bt][:, j, :],
                                in_=pss[bt][:],
                                func=AF.Copy,
                                scale=al,
                            )
                        else:
                            nc.vector.tensor_scalar_mul(
                                stages[bt][:, j, :], pss[bt][:], al
                            )
                for bt in range(2):
                    nc.sync.dma_start(
                        out=ovn[2 * c + bt][:, g * GSIZE : (g + 1) * GSIZE, :],
                        in_=stages[bt][:],
                    )
                # prep two pairs ahead, mid-pair to avoid bunching the
                # dequants against the pair boundary
                if g == 1 and c + 2 < NPAIRS:
                    prep(c + 2)

    nc.compile()
    _cache[key] = nc
    return nc


def run_sharded(x, y, az, bz, al, trace=False, tmpdir=None):
    """Shard inputs over 8 cores, run, gather. Returns (out, BassKernelResults)."""
    from concourse.bass_utils import run_bass_kernel_spmd

    nc = _build(az, bz, al)
    # host-side layout-only reorder: x[b, s, d] -> xT[b, d, r, p], s = 8p + r
    xT = np.ascontiguousarray(
        x.reshape(B, 128, 8, D).transpose(0, 3, 2, 1)
    )
    in_maps = [
        {
            "x": xT[i * BPC : (i + 1) * BPC],
            "y": y[i * BPC : (i + 1) * BPC],
        }
        for i in range(N_CORES)
    ]
    res = run_bass_kernel_spmd(
        nc, in_maps, list(range(N_CORES)), trace=trace, tmpdir=tmpdir
    )
    # device stores bf16; upcast to the contract f32 on the host
    out = np.empty((B, S, S), dtype=np.float32)
    for i, r in enumerate(res.results):
        out[i * BPC : (i + 1) * BPC] = r["out"]
    return out, res


def kernel(x, y, a_zp, b_zp, alpha):
    x = np.ascontiguousarray(np.asarray(x).astype(np.int8, copy=False))
    y = np.ascontiguousarray(np.asarray(y).astype(np.int8, copy=False))
    az = float(np.asarray(a_zp))
    bz = float(np.asarray(b_zp))
    al = float(np.asarray(alpha))
    out, _ = run_sharded(x, y, az, bz, al)
    return out


# revision 15
# speedup vs baseline: 1.1863x; 1.1863x over previous
"""Trainium2 Bass kernel: batched int8 dequant-BMM.

out[b] = (x[b].f32 - a_zp) @ (y[b].f32 - b_zp) * alpha
  x: [96, 1024, 64] int8, y: [96, 64, 1024] int8 -> out: [96, 1024, 1024] f32

Sharding: batch dim 96 -> 12 per core across 8 cores (pure data parallel).

The kernel is HBM-store-roofline bound: 12x1024x1024 output elems/core.
Output is alpha * K with K an exact integer < 2^21, so bf16 rounding of
the final value has rel err <= 2^-8 ~ 4e-3 (vs the 2e-2 gate): store
bf16 (25.2 MB/core, ~70us at the 358 GB/s HBM-per-NC limit) instead of
f32 and upcast on host.

The co-critical resource is PSUM->SBUF copy throughput - only ACT and
DVE can read PSUM (measured ~1.34us / ~1.47us per [128,1024] fp32
copy incl overhead; no 16-bit speedup possible with fp32 PSUM source,
and fp32 PSUM is mandatory on TRN2). 96 copies across 2 engines ~=
the store stream, so everything else is kept off ACT/DVE:
  - x is pre-transposed ON HOST (layout only) to [B, 64(d), 8(r),
    128(p)] so lhsT tiles come straight from DRAM - no on-device
    transpose pipeline at all.
  - pairs 1-5 load via SWDGE cast-DMA (int8 DRAM -> bf16 SBUF, cast in
    the DMA engine), so their zero-point subtract is a cheap all-bf16
    ACT activation (~0.6us); pair 0 loads via HWDGE (which starts ~1us
    earlier than SWDGE behind the ~7us engine preamble) as raw int8
    with 1x DVE dequants, to get the first store out ASAP.
  - GpSimd tensor ops were measured at 14.7us per dequant - unusable.
Steady state per pair (2 batches, 16 m-tile copies, 4.2 MB stored):
stores 11.7us, ACT 8 copies + 2 deqs ~= 11.9us, DVE 8 copies ~= 11.8us,
PE ~7us: copies and stores are balanced within ~2%. gsize=2 stores
(0.5 MB) keep the copy->store coupling slack small; 4 PSUM tiles
(8 banks) ring matmuls ahead of copies.
"""

import numpy as np

B, S, D = 96, 1024, 64
N_CORES = 8
BPC = B // N_CORES  # batches per core = 12
NPAIRS = BPC // 2

_cache = {}


def _build(az: float, bz: float, al: float):
    key = (az, bz, al)
    if key in _cache:
        return _cache[key]

    from contextlib import ExitStack

    import concourse.mybir as mybir
    import concourse.tile as tile
    from concourse import bacc

    f32 = mybir.dt.float32
    bf16 = mybir.dt.bfloat16
    i8 = mybir.dt.int8
    AF = mybir.ActivationFunctionType

    nc = bacc.Bacc(
        "TRN2", target_bir_lowering=False, debug=False, num_devices=N_CORES
    )
    # x arrives host-pre-transposed as [b, d, r, p] with s = 8p + r
    x_d = nc.dram_tensor("x", [BPC, D, 8, 128], i8, kind="ExternalInput").ap()
    y_d = nc.dram_tensor("y", [BPC, D, S], i8, kind="ExternalInput").ap()
    o_d = nc.dram_tensor("out", [BPC, S, S], bf16, kind="ExternalOutput").ap()

    # x[2c+bt, d, r, p] -> xv[bt*64+d, c, r, p]  (1KB runs per partition)
    xv = x_d.rearrange("(c b2) d r p -> (b2 d) c r p", b2=2)
    # y[2c+bt, d, s] -> yv[bt*64+d, c, s]  (contiguous in DRAM)
    yv = y_d.rearrange("(c b2) d s -> (b2 d) c s", b2=2)
    # out[b, 8p+r, t] <- ovn[b, p, r, t]: the row-residue m-tiling makes
    # the store rows of one partition contiguous in DRAM (gsize*2KB runs)
    ovn = o_d.rearrange("b (p r) t -> b p r t", p=128, r=8)

    GSIZE = 2  # r-tiles per store: small stores = tight store pipelining

    with tile.TileContext(nc) as tc, ExitStack() as ctx:
        xin_pool = ctx.enter_context(tc.tile_pool(name="xin", bufs=NPAIRS))
        yin_pool = ctx.enter_context(tc.tile_pool(name="yin", bufs=1))
        y0_pool = ctx.enter_context(tc.tile_pool(name="y0", bufs=1))
        xt_pool = ctx.enter_context(tc.tile_pool(name="xt", bufs=3))
        ybf_pool = ctx.enter_context(tc.tile_pool(name="ybf", bufs=3))
        stage_pool = ctx.enter_context(tc.tile_pool(name="stage", bufs=12))
        mpsum_pool = ctx.enter_context(
            tc.tile_pool(name="mpsum", bufs=4, space="PSUM")
        )

        # Pair 0 rides HWDGE as raw int8, first thing on the sync and
        # scalar queues - these issue right after the engine preamble
        # (~6us), about 2us before the gpsimd (SWDGE) queue gets going.
        y_sb = yin_pool.tile([128, NPAIRS, S], bf16)
        x0 = xin_pool.tile([128, 8, 128], i8, tag="x2")
        y0 = y0_pool.tile([128, S], i8)
        nc.sync.dma_start(out=x0[:], in_=xv[:, 0])
        nc.scalar.dma_start(out=y0[:], in_=yv[:, 0, :])
        x2s = [x0]

        # No PE warmup: warm matmuls at cold clock sit on the FIFO PE
        # queue ahead of the real stream (+2us to first store), and even
        # a 1.2 GHz PE (0.9us per m-tile fill pair) stays ahead of the
        # ~1.4us copies that gate the steady state.

        # Pairs 1-5 ride SWDGE cast-DMA (int8 DRAM -> bf16 SBUF; only
        # SWDGE casts) up front into the pre-store DMA window.
        for c in range(1, NPAIRS):
            x2 = xin_pool.tile([128, 8, 128], bf16, tag="x2")
            nc.gpsimd.dma_start(out=x2[:], in_=xv[:, c])
            nc.gpsimd.dma_start(out=y_sb[:, c, :], in_=yv[:, c, :])
            x2s.append(x2)

        # Zero-point subtract. Pair 0: int8 1x ops split across DVE and
        # ACT so they run in parallel before the copy stream starts.
        # Pairs 1-5: DVE bf16 tensor_scalar (4x perf mode, ~0.5us
        # measured; an ACT bf16 activation measured 1.37us - as costly
        # as a copy). The DVE 4x (2-port) deqs would starve SWDGE
        # descriptor generation, but all SWDGE loads complete by ~15us
        # and the first in-stream deq (prep(2)) issues after that.
        preps = {}

        def prep(c):
            xt = xt_pool.tile([128, 8, 128], bf16, tag="xt")
            y2bf = ybf_pool.tile([128, S], bf16, tag="y2bf")
            if c == 0:
                nc.vector.tensor_scalar_add(xt[:], x2s[0][:], -az)
                nc.scalar.activation(
                    out=y2bf[:], in_=y0[:], func=AF.Copy, bias=-bz, scale=1.0
                )
            else:
                nc.vector.tensor_scalar_add(xt[:], x2s[c][:], -az)
                nc.vector.tensor_scalar_add(y2bf[:], y_sb[:, c, :], -bz)
            preps[c] = (xt, y2bf)

        prep(0)
        prep(1)

        for c in range(NPAIRS):
            xt, y2bf = preps.pop(c)
            # e (bt=0, PE rows 0-63) and o (bt=1, rows 64-127) matmuls
            # issue adjacently so the row-tiled PE runs them concurrently.
            for g in range(8 // GSIZE):
                stages = []
                for bt in range(2):
                    stg = stage_pool.tile([128, GSIZE, S], bf16, tag="stage")
                    stages.append(stg)
                for j in range(GSIZE):
                    m = g * GSIZE + j
                    pss = []
                    for bt in range(2):
                        ps = mpsum_pool.tile([128, S], f32, tag="mpsum")
                        pss.append(ps)
                    for nh in range(2):
                        for bt in range(2):
                            nc.tensor.matmul(
                                pss[bt][:, nh * 512 : (nh + 1) * 512],
                                xt[bt * 64 : (bt + 1) * 64, m, :],
                                y2bf[bt * 64 : (bt + 1) * 64, nh * 512 : (nh + 1) * 512],
                                start=True,
                                stop=True,
                                tile_position=(bt * 64, 0),
                            )
                    # copies interleaved by parity; ACT takes a 9th on
                    # pairs where DVE carries the prep dequants
                    for bt in range(2):
                        k = m * 2 + bt
                        if k % 2 == 0 or (k == 7 and c < 4):
                            nc.scalar.activation(
                                out=stages[bt][:, j, :],
                                in_=pss[bt][:],
                                func=AF.Copy,
                                scale=al,
                            )
                        else:
                            nc.vector.tensor_scalar_mul(
                                stages[bt][:, j, :], pss[bt][:], al
                            )
                for bt in range(2):
                    nc.sync.dma_start(
                        out=ovn[2 * c + bt][:, g * GSIZE : (g + 1) * GSIZE, :],
                        in_=stages[bt][:],
                    )
                # prep two pairs ahead, mid-pair to avoid bunching the
                # dequants against the pair boundary
                if g == 1 and c + 2 < NPAIRS:
                    prep(c + 2)

    nc.compile()
    _cache[key] = nc
    return nc


def run_sharded(x, y, az, bz, al, trace=False, tmpdir=None):
    """Shard inputs over 8 cores, run, gather. Returns (out, BassKernelResults)."""
    from concourse.bass_utils import run_bass_kernel_spmd

    nc = _build(az, bz, al)
    # host-side layout-only reorder: x[b, s, d] -> xT[b, d, r, p], s = 8p + r
    xT = np.ascontiguousarray(
        x.reshape(B, 128, 8, D).transpose(0, 3, 2, 1)
    )
    in_maps = [
        {
            "x": xT[i * BPC : (i + 1) * BPC],
            "y": y[i * BPC : (i + 1) * BPC],
        }
        for i in range(N_CORES)
    ]
    res = run_bass_kernel_spmd(
        nc, in_maps, list(range(N_CORES)), trace=trace, tmpdir=tmpdir
    )
    # device stores bf16; upcast to the contract f32 on the host
    out = np.empty((B, S, S), dtype=np.float32)
    for i, r in enumerate(res.results):
        out[i * BPC : (i + 1) * BPC] = r["out"]
    return out, res


def kernel(x, y, a_zp, b_zp, alpha):
    x = np.ascontiguousarray(np.asarray(x).astype(np.int8, copy=False))
    y = np.ascontiguousarray(np.asarray(y).astype(np.int8, copy=False))
    az = float(np.asarray(a_zp))
    bz = float(np.asarray(b_zp))
    al = float(np.asarray(alpha))
    out, _ = run_sharded(x, y, az, bz, al)
    return out
